# revision 9
# baseline (speedup 1.0000x reference)
# Trainium2 Bass kernel for nn_ChannelTail (channel self-attention tail).
#
# Math (per batch element b):
#   value = w_value @ x_b + b_value            [256, HW]
#   A     = softmax(energy_b, axis=-1)         [256, 256]
#   out   = w_re @ (A @ value) + b_re          [512, HW]
#   y     = gamma * out + 2 * x_b
#
# Full algebraic fusion into ONE pixel GEMM: with
#   M'       = gamma * w_re @ A @ w_value                [512, 512]
#   bias_tot = gamma * (w_re @ A @ b_value + b_re)       [512]
# we have  y = M' @ x_b + bias_tot + 2*x_b.  M' and bias_tot are computed
# once per core from softmax(energy) via small on-device GEMMs (~100
# MMACs, ~2% of the pixel GEMM).
#
# Per-core engine budget per pass (measured by microbenchmark):
#   DMA  ~104 us  <- bottleneck: x in bf16 (16.8 MB) + y out bf16 (16.8 MB)
#                   at ~324 GB/s combined (per-NC HBM limit ~358)
#   PE   ~64 us   fp8 DoubleRow GEMM: 256 MMs [K=256(x2-packed),128]@[.,512]
#   ACT  ~72 us   PSUM drain + bias (scale=1/64 undoes fp8 weight scaling)
#   DVE  ~64 us   bf16->fp8 cast of x (~11 us) + epilogue y = 2x + t
#
# Why fp8 DoubleRow for the GEMM (both operands fp8, 2 MACs/cell/cycle):
#  - bf16 matmuls cost ~274 ns per [128,128]@[128,512] on HW (LDWEIGHTS
#    is never hidden for full-array matmuls and walrus never dedupes it;
#    N>512 fails the ISA check) -> 512 MMs = 140 us, above the DMA floor.
#    DoubleRow halves the MM count and streams 2 rows/cycle: ~72 us.
#  - accuracy: the attention branch |gamma*W2@x| <= ~0.034 while the
#    output is dominated by 2x (scale ~10.8), so fp8 error on the branch
#    is invisible: measured rel err 4.5e-3, same as an all-bf16 kernel
#    (bf16 rounding of x and y dominates). Weights are scaled x64 into
#    fp8's normal range; the output drain applies scale=1/64.
#  - the "+2x" epilogue stays exact-ish via bf16 x on the DVE.
#
# Sharding: data-parallel over batch. 8 batch elements, 8 cores, one
# batch element per core. Weights replicated. No collectives.

import numpy as np
from contextlib import ExitStack

B, C_IN, C_INT, H, W = 8, 512, 256, 128, 128
HW = H * W            # 16384
NT = 512              # pixels per compute sub-tile (one PSUM bank fp32)
NCORES = 8
P = 128               # partitions
KI = C_IN // P        # 4 input/output-channel chunks (512)
KM = C_INT // P       # 2 intermediate-channel chunks (256)
WSCALE = 64.0         # fp8 weight pre-scale (undone in the output drain)

_built = None


def _build(reps=1):
    """Trace + schedule + compile the Bass program. Returns nc.

    reps>1 repeats the main pixel loop (same data) for benchmarking:
    steady-state time per rep = (t(R2)-t(R1))/(R2-R1).
    """
    import concourse.bass as bass
    import concourse.mybir as mybir
    import concourse.tile as tile
    from concourse import bacc
    from concourse.bass import ds

    fp32 = mybir.dt.float32
    bf16 = mybir.dt.bfloat16
    fp8 = mybir.dt.float8e4
    # x super-tile: 4096 px = 8KB contiguous runs per row (bf16).
    SUP = 4096
    N_SUP = HW // SUP
    SUBS = SUP // NT
    AF = mybir.ActivationFunctionType
    OP = mybir.AluOpType
    AX = mybir.AxisListType
    DR = mybir.MatmulPerfMode.DoubleRow

    nc = bacc.Bacc("TRN2", target_bir_lowering=False, debug=False,
                   num_devices=NCORES)

    energy = nc.dram_tensor("energy", [C_INT, C_INT], fp32, kind="ExternalInput").ap()
    x_d = nc.dram_tensor("x", [C_IN, HW], bf16, kind="ExternalInput").ap()
    wval_d = nc.dram_tensor("w_value_n", [C_INT, C_IN], bf16, kind="ExternalInput").ap()
    wrT_d = nc.dram_tensor("w_reT", [C_INT, C_IN], bf16, kind="ExternalInput").ap()
    bvc_d = nc.dram_tensor("b_value_c", [P, KM], bf16, kind="ExternalInput").ap()
    bre_d = nc.dram_tensor("b_re_t", [P, KI], fp32, kind="ExternalInput").ap()
    gam_d = nc.dram_tensor("gamma", [1, 1], fp32, kind="ExternalInput").ap()
    out_d = nc.dram_tensor("out", [C_IN, HW], bf16, kind="ExternalOutput").ap()

    # chunked DRAM views: row (q*128 + p) -> [p, q, cols]
    xv = x_d.rearrange("(q p) n -> p q n", p=P)     # [128, 4, HW]
    ov = out_d.rearrange("(q p) n -> p q n", p=P)   # [128, 4, HW]

    with tile.TileContext(nc) as tc, ExitStack() as ctx:
        const = ctx.enter_context(tc.tile_pool(name="const", bufs=1))

        # ---------- load constants (SWDGE; setup only) ----------
        e_sb = []
        for i in range(KM):
            t = const.tile([P, C_INT], fp32, tag=f"e{i}", name=f"e{i}")
            nc.gpsimd.dma_start(t[:], energy.rearrange("(k p) m -> k p m", p=P)[i])
            e_sb.append(t)
        wval_sb = []
        for k in range(KM):
            t = const.tile([P, C_IN], bf16, tag=f"wval{k}", name=f"wval{k}")
            nc.gpsimd.dma_start(t[:], wval_d.rearrange("(k p) m -> k p m", p=P)[k])
            wval_sb.append(t)
        wrT_sb = []
        for k in range(KM):
            t = const.tile([P, C_IN], bf16, tag=f"wrT{k}", name=f"wrT{k}")
            nc.gpsimd.dma_start(t[:], wrT_d.rearrange("(k p) m -> k p m", p=P)[k])
            wrT_sb.append(t)
        bvc_sb = const.tile([P, KM], bf16, tag="bvc")
        nc.gpsimd.dma_start(bvc_sb[:], bvc_d)
        bre_sb = const.tile([P, KI], fp32, tag="bre")
        nc.gpsimd.dma_start(bre_sb[:], bre_d)
        g_bc = const.tile([P, 1], fp32, tag="gbc")
        nc.gpsimd.dma_start(g_bc[:], gam_d.to_broadcast([P, 1]))

        # bias2 = gamma * b_re   [128, 4]
        bias2 = const.tile([P, KI], fp32, tag="bias2")
        nc.vector.tensor_scalar_mul(bias2[:], bre_sb[:], g_bc[:])

        # ---------- A_g = gamma * softmax(energy)  (bf16) ----------
        Ag_sb = []
        for i in range(KM):
            negmax = const.tile([P, 1], fp32, tag=f"negmax{i}", name=f"negmax{i}")
            nc.vector.tensor_reduce(negmax[:], e_sb[i][:], axis=AX.X, op=OP.max,
                                    negate=True)
            pexp = const.tile([P, C_INT], fp32, tag=f"pexp{i}", name=f"pexp{i}")
            sums = const.tile([P, 1], fp32, tag=f"sums{i}", name=f"sums{i}")
            nc.scalar.activation(pexp[:], e_sb[i][:], AF.Exp, bias=negmax[:],
                                 scale=1.0, accum_out=sums[:])
            rec = const.tile([P, 1], fp32, tag=f"rec{i}", name=f"rec{i}")
            nc.vector.reciprocal(rec[:], sums[:])
            recg = const.tile([P, 1], fp32, tag=f"recg{i}", name=f"recg{i}")
            nc.vector.tensor_scalar_mul(recg[:], rec[:], g_bc[:])
            a = const.tile([P, C_INT], bf16, tag=f"A{i}", name=f"A{i}")
            nc.vector.tensor_scalar_mul(a[:], pexp[:], recg[:])
            Ag_sb.append(a)

        # ---------- small GEMM chain for M'.T (fp8, x64) and bias_tot ----
        # T1 = A_g.T @ w_reT = (gamma * w_re @ A).T            [256, 512]
        # M'.T = w_valueT @ T1   -> drained x64 into fp8       [512, 512]
        # bias_tot = T1.T @ b_value + gamma*b_re               [512]
        # WT8[j][p, i, o] = 64 * M'.T[(2j+i)*128+p, o]  (DoubleRow layout)
        WT8 = []
        for j in range(KI // 2):
            t = const.tile([P, 2, C_IN], fp8, tag=f"WT8{j}", name=f"WT8{j}")
            WT8.append(t)
        btot = const.tile([P, KI], fp32, tag="btot")
        with tc.tile_pool(name="psum_setup", bufs=2, space="PSUM") as psum_setup:
            T1_sb = []
            for i in range(KM):
                ps = psum_setup.tile([P, C_IN], fp32, tag="t1_ps")
                for k in range(KM):
                    nc.tensor.matmul(ps[:],
                                     Ag_sb[k][:, i * P:(i + 1) * P],
                                     wrT_sb[k][:],
                                     start=(k == 0), stop=(k == KM - 1))
                t1 = const.tile([P, C_IN], bf16, tag=f"T1_{i}", name=f"T1_{i}")
                nc.scalar.activation(t1[:], ps[:], AF.Identity, scale=1.0)
                T1_sb.append(t1)

            for mo in range(KI):
                ps2 = psum_setup.tile([P, C_IN], fp32, tag="wt_ps")
                for k in range(KM):
                    nc.tensor.matmul(ps2[:],
                                     wval_sb[k][:, mo * P:(mo + 1) * P],
                                     T1_sb[k][:],
                                     start=(k == 0), stop=(k == KM - 1))
                nc.scalar.activation(WT8[mo // 2][:, mo % 2, :], ps2[:],
                                     AF.Identity, scale=WSCALE)

                psb = psum_setup.tile([P, 1], fp32, tag="bt_ps")
                for k in range(KM):
                    nc.tensor.matmul(psb[:],
                                     T1_sb[k][:, mo * P:(mo + 1) * P],
                                     bvc_sb[:, k:k + 1],
                                     start=(k == 0), stop=(k == KM - 1))
                nc.scalar.activation(btot[:, mo:mo + 1], psb[:], AF.Identity,
                                     bias=bias2[:, mo:mo + 1], scale=1.0)

        # ---------- main loop over pixel super-tiles ----------
        px = ctx.enter_context(tc.tile_pool(name="px", bufs=2))
        px8 = ctx.enter_context(tc.tile_pool(name="px8", bufs=2))
        pt = ctx.enter_context(tc.tile_pool(name="pt", bufs=6))
        pout = ctx.enter_context(tc.tile_pool(name="pout", bufs=2))
        ps_out = ctx.enter_context(tc.tile_pool(name="ps_out", bufs=8, space="PSUM"))

        for s in range(N_SUP * reps):
            s = s % N_SUP
            x_t = px.tile([P, KI, SUP], bf16, tag="x")
            nc.scalar.dma_start(x_t[:], xv[:, :, ds(s * SUP, SUP)])
            x8 = px8.tile([P, KI, SUP], fp8, tag="x8")
            nc.vector.tensor_copy(
                x8.rearrange("p q n -> p (q n)")[:],
                x_t.rearrange("p q n -> p (q n)")[:])
            out_t = pout.tile([P, KI, SUP], bf16, tag="out")

            for u in range(SUBS):
                lo = u * NT
                for mo in range(KI):
                    po = ps_out.tile([P, NT], fp32, tag="po")
                    for j in range(KI // 2):
                        nc.tensor.matmul(
                            po[:],
                            WT8[j][:, :, mo * P:(mo + 1) * P],
                            x8[:, 2 * j:2 * j + 2, lo:lo + NT],
                            start=(j == 0), stop=(j == KI // 2 - 1),
                            perf_mode=DR)
                    t = pt.tile([P, NT], bf16, tag="t")
                    nc.scalar.activation(t[:], po[:], AF.Identity,
                                         bias=btot[:, mo:mo + 1],
                                         scale=1.0 / WSCALE)
                    nc.vector.scalar_tensor_tensor(
                        out_t[:, mo, lo:lo + NT],
                        x_t[:, mo, lo:lo + NT], 2.0, t[:],
                        op0=OP.mult, op1=OP.add)

            # whole-super store (4MiB bf16) on the SP HWDGE ring
            nc.sync.dma_start(ov[:, :, ds(s * SUP, SUP)], out_t[:])

    nc.compile()
    return nc


def _get_built(reps=1):
    global _built
    if _built is None:
        _built = {}
    if reps not in _built:
        _built[reps] = _build(reps)
    return _built[reps]


def _prep_in_maps(energy, x, w_value, b_value, w_re, b_re, gamma):
    import ml_dtypes
    bf = ml_dtypes.bfloat16
    wval = np.ascontiguousarray(np.asarray(w_value, np.float32)).astype(bf)
    wrT = np.ascontiguousarray(np.asarray(w_re, np.float32).T).astype(bf)
    bvc = np.ascontiguousarray(
        np.asarray(b_value, np.float32).reshape(KM, P).T).astype(bf)
    bre_t = np.ascontiguousarray(np.asarray(b_re, np.float32).reshape(KI, P).T)
    gam = np.asarray(gamma, np.float32).reshape(1, 1)
    x = np.asarray(x, np.float32)
    energy = np.asarray(energy, np.float32)

    in_maps = []
    for b in range(NCORES):
        in_maps.append({
            "energy": np.ascontiguousarray(energy[b]),
            "x": np.ascontiguousarray(x[b].reshape(C_IN, HW)).astype(bf),
            "w_value_n": wval,
            "w_reT": wrT,
            "b_value_c": bvc,
            "b_re_t": bre_t,
            "gamma": gam,
        })
    return in_maps


def run(inputs, trace=False, **kw):
    """Run on 8 cores; returns (output [B,C_IN,H,W], BassKernelResults)."""
    from concourse.bass_utils import run_bass_kernel_spmd
    nc = _get_built()
    in_maps = _prep_in_maps(**inputs)
    res = run_bass_kernel_spmd(nc, in_maps, core_ids=list(range(NCORES)),
                               trace=trace, **kw)
    out = np.stack([np.asarray(r["out"], np.float32) for r in res.results])
    return out.reshape(B, C_IN, H, W), res


def kernel(**inputs) -> np.ndarray:
    out, _ = run(inputs, trace=False)
    return out
